# revision 33
# baseline (speedup 1.0000x reference)
"""AttentionPooling (segment softmax + weighted segment sum) on 8 trn2 cores.

Math (per graph g): out[g] = sum_n softmax_g(s)_n * x[n] over nodes n with
batch[n] == g, where s = tanh(x @ W1 + b1) @ W2 + b2.

Key design points:
  * exp(s) cannot overflow fp32 -> accumulate unnormalized exp(s)*x and
    exp(s), divide once at the end.  b2 shifts every score equally and
    cancels in the softmax -> dropped entirely.
  * batch is sorted, so sharding by graph (128 graphs per core) gives each
    core one contiguous node range: pure data parallel, no collectives.
  * Pool = matmul with weighted one-hot lhsT st[n, g'] = e_n * (bl[n] == g')
    over a 32-graph window (M=32).  The 4 blocks of a chunk go to four
    DIFFERENT tile_position col groups (slot = (window + lane) % 4, one PSUM
    accumulator per lane) so they stream CONCURRENTLY on the PE's 32-col
    sub-arrays (~284ns for 4 blocks vs ~548ns serialized).  The final
    combine un-rotates with 7 partition-shifted DVE ops.
  * Scores are written at PASS-aligned PSUM columns (a block covering two
    windows emits its score twice - only ~9 extra N=1 matmuls total), so
    the whole one-hot build for a chunk is TWO DVE tensor_tensor ops with
    3D broadcast APs (is_equal vs bcols, multiply by ee) instead of ~9
    per-pass ops: DVE fixed overhead (~90-130ns/op) dominated the v1 build.
  * ONE Exp per super-chunk: ScalarE ACTIVATE costs (N+352)/1.2 ns, so
    batching 16+ scores per exp amortizes the 352-cycle fixed cost.
  * ~22 N=512 warmup matmuls on zeroed data keep the PE busy through the
    HAM activity window (~3.4us) during the initial DMA fill, so the clock
    gate is at 8/8 (2.4 GHz) when real work starts.
  * Both x streams are fp8 e3m4; W1 stays bf16 (rel err 1.46e-2 < 2e-2).
"""

import sys
from contextlib import ExitStack

import numpy as np

for _p in ("/opt/trn_rl_repo",):
    if _p not in sys.path:
        sys.path.insert(0, _p)

import ml_dtypes

import concourse.bass as bass
import concourse.bacc as bacc
import concourse.tile as tile
from concourse import mybir

N_NODES = 500_000
HIDDEN = 256
NUM_GRAPHS = 1024
N_CORES = 8
G_LOC = NUM_GRAPHS // N_CORES  # 128 graphs per core == PSUM partition dim
H = HIDDEN // 2  # 128 hidden units in the attention MLP
BLK = 128  # nodes per block (matmul contraction tile)
NBPC = 4  # blocks per chunk (also: pool rotation lanes)
CH = BLK * NBPC  # 512 nodes per compute chunk (one PSUM bank at fp32)
CPS = 4  # compute chunks per DMA super-chunk
SUP = CH * CPS  # 2048 nodes per DMA (~1 MB per stream -> efficient descriptors)
WIN = 32  # pool window: graphs per one-hot / PSUM col group
NLANE = 4  # pool rotation lanes == NBPC
BF16 = mybir.dt.bfloat16
E3M4 = mybir.dt.float8e3  # 4 mantissa bits: x streams (rel err ~3%, max ~15.5)
F32 = mybir.dt.float32

_PROGRAM_CACHE: dict = {}


def build_program(n_pad: int, passes: tuple, use_b1: bool) -> bass.Bass:
    """passes[blk] = tuple of 32-graph windows the block's pool matmul must
    cover (union across cores; usually 1, occasionally 2)."""
    assert n_pad % SUP == 0
    nblk = n_pad // BLK
    nsup = n_pad // SUP
    nchunks = n_pad // CH
    assert len(passes) == nblk

    # flat pass list [(blk, w, idx)] in emission order; per (lane, slot) the
    # first and last flat index (lane = blk % NLANE, slot = (w+lane) % NLANE)
    flat = []
    for blk in range(nblk):
        for w in passes[blk]:
            flat.append((blk, w, len(flat)))
    npass = len(flat)
    first = {}
    last = {}
    for blk, w, idx in flat:
        lane = blk % NLANE
        slot = (w + lane) % NLANE
        first.setdefault((lane, slot), idx)
        last[(lane, slot)] = idx
    pass_of_blk = {}
    for blk, w, idx in flat:
        pass_of_blk.setdefault(blk, []).append((w, idx))

    # per-chunk / per-super pass spans (flat indices are contiguous per chunk)
    def blk_range_passes(b0, b1):
        return [
            (blk, w, idx)
            for blk, w, idx in flat
            if b0 <= blk < b1
        ]

    chunk_passes = [blk_range_passes(t * NBPC, (t + 1) * NBPC) for t in range(nchunks)]
    sup_start = []
    for s in range(nsup):
        sp_list = blk_range_passes(s * NBPC * CPS, (s + 1) * NBPC * CPS)
        sup_start.append(sp_list[0][2] if sp_list else npass)
    sup_npass = [
        len(blk_range_passes(s * NBPC * CPS, (s + 1) * NBPC * CPS))
        for s in range(nsup)
    ]
    maxpc = max((len(cp) for cp in chunk_passes), default=1)
    supw = max(sup_npass) if sup_npass else 1
    assert supw <= WIN, f"super pass count {supw} exceeds sp tile width"

    nc = bacc.Bacc("TRN2")
    # host-swizzled so each super-chunk DMA reads one contiguous ~4KB run per
    # partition: xaug[s, p, b, f] = [x | 1.0][s*SUP + b*BLK + p, f]
    xaug = nc.dram_tensor(
        "xaug", [nsup, BLK, NBPC * CPS, HIDDEN + 1], E3M4, kind="ExternalInput"
    )
    # xT[s, p, j, n] = x[s*SUP + n, BLK*j + p], fp8: feeds only the score MLP
    xT = nc.dram_tensor("xT", [nsup, BLK, 2, SUP], E3M4, kind="ExternalInput")
    # bcols[p, pass] = batch_local[blk(pass)*BLK + p] - 32*w(pass)  (or pad)
    bcols = nc.dram_tensor("bcols", [BLK, max(npass, 1)], BF16, kind="ExternalInput")
    # w1[p, j, h] = W1[BLK*j + p, h]
    w1 = nc.dram_tensor("w1", [BLK, 2, H], BF16, kind="ExternalInput")
    w2 = nc.dram_tensor("w2", [H, 1], BF16, kind="ExternalInput")
    if use_b1:
        b1 = nc.dram_tensor("b1", [H, 1], F32, kind="ExternalInput")
    # raw rotated lane accumulators; the host un-rotates, sums lanes and
    # normalizes (cheap numpy) - saves ~5us of on-device tail work
    out = nc.dram_tensor(
        "out", [G_LOC, NLANE, HIDDEN + 1], F32, kind="ExternalOutput"
    )

    with tile.TileContext(nc) as tc, ExitStack() as ctx:
        singles = ctx.enter_context(tc.tile_pool(name="singles", bufs=1))
        xa_pool = ctx.enter_context(tc.tile_pool(name="xa", bufs=4))
        xt_pool = ctx.enter_context(tc.tile_pool(name="xt", bufs=4))
        tt_pool = ctx.enter_context(tc.tile_pool(name="tt", bufs=3))
        oh_pool = ctx.enter_context(tc.tile_pool(name="oh", bufs=4))
        st_pool = ctx.enter_context(tc.tile_pool(name="st", bufs=4))
        ee_pool = ctx.enter_context(tc.tile_pool(name="ee", bufs=2))
        hp_pool = ctx.enter_context(tc.tile_pool(name="hp", bufs=2, space="PSUM"))
        sp_pool = ctx.enter_context(tc.tile_pool(name="sp", bufs=2, space="PSUM"))
        acc_pool = ctx.enter_context(tc.tile_pool(name="acc", bufs=1, space="PSUM"))

        # singles go through ScalarE's HWDGE trigger queue so SyncE's first
        # (serial, ~600ns each) triggers are the xt[0] quarter fills
        w1_sb = singles.tile([BLK, 2, H], BF16)
        nc.scalar.dma_start(out=w1_sb, in_=w1[:, :, :])
        w2_sb = singles.tile([H, 1], BF16)
        nc.scalar.dma_start(out=w2_sb, in_=w2[:, :])
        bc_sb = singles.tile([BLK, max(npass, 1)], BF16)
        nc.scalar.dma_start(out=bc_sb, in_=bcols[:, :])
        if use_b1:
            b1_sb = singles.tile([H, 1], F32)
            nc.scalar.dma_start(out=b1_sb, in_=b1[:, :])
        # junk (warmup operand) is filled by a gpsimd iota, the earliest
        # available producer (~2.9us; the fp8 gpsimd memset used before took
        # until ~8us, and the DVE stream is blocked by its table-load DMA)
        junk = singles.tile([BLK, CH], BF16)
        nc.gpsimd.iota(
            out=junk,
            pattern=[[1, CH]],
            base=0,
            channel_multiplier=0,
            allow_small_or_imprecise_dtypes=True,
        )
        iota_sb = singles.tile([BLK, WIN], BF16)
        nc.gpsimd.iota(
            out=iota_sb,
            pattern=[[1, WIN]],
            base=0,
            channel_multiplier=0,
            allow_small_or_imprecise_dtypes=True,
        )

        # rotated pool accumulators: lane j accumulates window w at partition
        # slot 32*((w+j)%4) of accs[j]
        accs = [
            acc_pool.tile([G_LOC, HIDDEN + 1], F32, tag=f"acc{j}", name=f"acc{j}")
            for j in range(NLANE)
        ]
        # zero any (lane, slot) region no matmul will ever write (the combine
        # below reads whole accumulators)
        for j in range(NLANE):
            for s in range(NLANE):
                if (j, s) not in first:
                    nc.vector.memset(accs[j][WIN * s : WIN * (s + 1), :], 0.0)

        # ~36 N=512 warmup matmuls (~8us) keep the PE busy through the HAM
        # activity window while the first super-chunk DMAs land.  They only
        # depend on the DVE memset above, so they start at ~0.4us.
        warm = hp_pool.tile([H, CH], F32, tag="hp", name="hp_warm")
        for i in range(36):
            nc.tensor.matmul(
                warm[0:WIN, :],
                lhsT=junk[:, 0:WIN],
                rhs=junk,
                start=True,
                stop=True,
            )

        xa_tiles = {}
        xt_tiles = {}
        tt_tiles = {}
        sp_tiles = {}
        ee_tiles = {}
        st_tiles = {}

        def emit_mlp(t):
            if not chunk_passes[t]:
                return
            s, q = divmod(t, CPS)
            xt = xt_tiles[s]
            hp = hp_pool.tile([H, CH], F32, tag="hp", name="hp")
            nc.tensor.matmul(
                hp,
                lhsT=w1_sb[:, 0, :],
                rhs=xt[:, 0, q * CH : (q + 1) * CH],
                start=True,
                stop=False,
            )
            nc.tensor.matmul(
                hp,
                lhsT=w1_sb[:, 1, :],
                rhs=xt[:, 1, q * CH : (q + 1) * CH],
                start=False,
                stop=True,
            )
            tt = tt_pool.tile([H, CH], E3M4, name="tt")
            kw = {"bias": b1_sb} if use_b1 else {}
            nc.scalar.activation(
                out=tt, in_=hp, func=mybir.ActivationFunctionType.Tanh, **kw
            )
            tt_tiles[t] = tt

        def emit_scores(t):
            if t not in tt_tiles:
                return
            s, q = divmod(t, CPS)
            if s not in sp_tiles:
                sp_tiles[s] = sp_pool.tile([BLK, WIN], F32, tag="sp", name="sp")
            sp = sp_tiles[s]
            tt = tt_tiles.pop(t)
            for blk, w, idx in chunk_passes[t]:
                b = blk % NBPC
                c = idx - sup_start[s]
                nc.tensor.matmul(
                    sp[:, c : c + 1],
                    lhsT=tt[:, b * BLK : (b + 1) * BLK],
                    rhs=w2_sb,
                    start=True,
                    stop=True,
                )

        def emit_exp(s):
            if s not in sp_tiles:
                return
            sp = sp_tiles.pop(s)
            n = sup_npass[s]
            ee = ee_pool.tile([BLK, WIN], BF16, tag="ee", name="ee")
            nc.scalar.activation(
                out=ee[:, 0:n], in_=sp[:, 0:n], func=mybir.ActivationFunctionType.Exp
            )
            ee_tiles[s] = ee

        def emit_st(t):
            """Two batched DVE ops build all weighted one-hots of chunk t."""
            cp = chunk_passes[t]
            if not cp:
                return
            s = t // CPS
            ee = ee_tiles[s]
            npc = len(cp)
            i0 = cp[0][2]
            j0 = i0 - sup_start[s]
            oh = oh_pool.tile([BLK, maxpc, WIN], BF16, tag="oh", name="oh")
            nc.vector.tensor_tensor(
                out=oh[:, 0:npc, :],
                in0=iota_sb[:, :].unsqueeze(1).broadcast_to((BLK, npc, WIN)),
                in1=bc_sb[:, i0 : i0 + npc].unsqueeze(2).broadcast_to((BLK, npc, WIN)),
                op=mybir.AluOpType.is_equal,
            )
            st = st_pool.tile([BLK, maxpc, WIN], BF16, tag="st", name="st")
            nc.vector.tensor_tensor(
                out=st[:, 0:npc, :],
                in0=oh[:, 0:npc, :],
                in1=ee[:, j0 : j0 + npc].unsqueeze(2).broadcast_to((BLK, npc, WIN)),
                op=mybir.AluOpType.mult,
            )
            st_tiles[t] = st

        def emit_pool(t):
            cp = chunk_passes[t]
            if not cp:
                return
            s, q = divmod(t, CPS)
            xa = xa_tiles[s]
            if q == CPS - 1:
                xa_tiles.pop(s)
            st = st_tiles.pop(t)
            i0 = cp[0][2]
            for blk, w, idx in cp:
                lane = blk % NLANE
                slot = (w + lane) % NLANE
                nc.tensor.matmul(
                    accs[lane][WIN * slot : WIN * (slot + 1), :],
                    lhsT=st[:, idx - i0, :],
                    rhs=xa[:, blk % (NBPC * CPS), :],
                    start=(idx == first[(lane, slot)]),
                    stop=(idx == last[(lane, slot)]),
                    tile_position=(0, WIN * slot),
                )

        def dma_xt(s, split=False):
            xt = xt_pool.tile([BLK, 2, SUP], E3M4, name="xt")
            if split:
                # finer first fills so the MLP can start sooner
                for qq in range(CPS):
                    nc.sync.dma_start(
                        out=xt[:, :, qq * CH : (qq + 1) * CH],
                        in_=xT[s][:, :, qq * CH : (qq + 1) * CH],
                    )
            else:
                nc.sync.dma_start(out=xt, in_=xT[s])
            xt_tiles[s] = xt

        def dma_xa(s):
            xa = xa_pool.tile([BLK, NBPC * CPS, HIDDEN + 1], E3M4, name="xa")
            nc.sync.dma_start(out=xa, in_=xaug[s])
            xa_tiles[s] = xa

        for t in range(nchunks + 6):
            s, q = divmod(t, CPS)
            if t == 0:
                # prioritize the MLP's stream: xt[0] (split), xt[1], THEN
                # xa[0] (first needed 6 slots later) - the DMA queue is FIFO
                # and the ramp is bandwidth-bound.
                dma_xt(0, split=True)
                if nsup > 1:
                    dma_xt(1)
                dma_xa(0)
            elif q == 0 and 1 <= s < nsup:
                if s + 1 < nsup:
                    dma_xt(s + 1)
                dma_xa(s)
            if t < nchunks:
                emit_mlp(t)
            if 0 <= t - 6 < nchunks:
                emit_pool(t - 6)
            if 0 <= t - 1 < nchunks:
                emit_scores(t - 1)
                if (t - 1) % CPS == CPS - 1:
                    emit_exp((t - 1) // CPS)
                    xt_tiles.pop((t - 1) // CPS, None)
            if 0 <= t - 4 < nchunks:
                emit_st(t - 4)

        # copy each (rotated) lane accumulator to SBUF (DVE/ScalarE split for
        # parallelism) and DMA all four out in ONE transfer (a dma_start
        # trigger costs ~620ns on SyncE); the host un-rotates + normalizes.
        accsb = singles.tile([G_LOC, NLANE, HIDDEN + 1], F32)
        for j in range(NLANE):
            if j % 2 == 0:
                nc.vector.tensor_copy(out=accsb[:, j, :], in_=accs[j])
            else:
                nc.scalar.copy(out=accsb[:, j, :], in_=accs[j])
        nc.sync.dma_start(out=out[:, :, :], in_=accsb)

    nc.finalize()
    return nc


def make_in_maps(x, batch, W1, b1, W2, b2):
    """Shard by graph (128 contiguous graphs per core), pad node counts to a
    common multiple of SUP, and lay out the per-core device arrays.  Also
    derives the uniform (across cores) pool pass structure."""
    x = np.asarray(x, dtype=np.float32)
    batch = np.asarray(batch)
    bounds = np.searchsorted(batch, np.arange(0, NUM_GRAPHS + 1, G_LOC))
    n_loc_max = int(np.diff(bounds).max())
    n_pad = max(SUP, ((n_loc_max + SUP - 1) // SUP) * SUP)
    nblk = n_pad // BLK

    # local (per-core) batch ids, -1 padding
    bl_all = np.full((N_CORES, n_pad), -1.0, np.float32)
    for c in range(N_CORES):
        s, e = int(bounds[c]), int(bounds[c + 1])
        bl_all[c, : e - s] = batch[s:e].astype(np.float32) - np.float32(c * G_LOC)

    # uniform pass structure: per block, union of windows over cores
    passes = []
    for blk in range(nblk):
        seg = bl_all[:, blk * BLK : (blk + 1) * BLK]
        ws = sorted({int(g) // WIN for g in np.unique(seg) if g >= 0})
        passes.append(tuple(ws))
    passes = tuple(passes)

    flat = [(blk, w) for blk in range(nblk) for w in passes[blk]]
    npass = len(flat)

    # w1[p, j, h] = W1[BLK*j + p, h], bf16 (scores must stay clean: the e3m4
    # pool stream eats most of the error budget)
    w1_8 = np.ascontiguousarray(
        np.asarray(W1, np.float32)
        .astype(ml_dtypes.bfloat16)
        .reshape(2, BLK, H)
        .transpose(1, 0, 2)
    )
    w2_bf = np.asarray(W2, np.float32).reshape(H, 1).astype(ml_dtypes.bfloat16)
    b1_f = np.asarray(b1, np.float32).reshape(H, 1)
    use_b1 = bool(np.any(b1_f != 0.0))

    in_maps = []
    for c in range(N_CORES):
        s, e = int(bounds[c]), int(bounds[c + 1])
        nloc = e - s
        xs = x[s:e]
        nsup = n_pad // SUP
        nb = NBPC * CPS
        xa = np.zeros((n_pad, HIDDEN + 1), ml_dtypes.float8_e3m4)
        xa[:nloc, :HIDDEN] = xs.astype(ml_dtypes.float8_e3m4)
        xa[:nloc, HIDDEN] = 1.0
        # [s*SUP + b*BLK + p, f] -> [s, p, b, f]
        xa = np.ascontiguousarray(
            xa.reshape(nsup, nb, BLK, HIDDEN + 1).transpose(0, 2, 1, 3)
        )
        # [s, p, j, n] = x[s*SUP + n, BLK*j + p]
        xT = np.zeros((HIDDEN, n_pad), ml_dtypes.float8_e3m4)
        xT[:, :nloc] = xs.T.astype(ml_dtypes.float8_e3m4)
        xT = np.ascontiguousarray(xT.reshape(2, BLK, nsup, SUP).transpose(2, 1, 0, 3))
        bl = bl_all[c]
        bcols = np.full((BLK, max(npass, 1)), -1e9, np.float32)
        for i, (blk, w) in enumerate(flat):
            bcols[:, i] = bl[blk * BLK : (blk + 1) * BLK] - np.float32(WIN * w)
        im = {
            "xaug": xa,
            "xT": xT,
            "bcols": np.ascontiguousarray(bcols.astype(ml_dtypes.bfloat16)),
            "w1": w1_8,
            "w2": w2_bf,
        }
        if use_b1:
            im["b1"] = b1_f
        in_maps.append(im)
    return in_maps, n_pad, passes, use_b1


def kernel(x, batch, W1, b1, W2, b2):
    from concourse.bass_utils import run_bass_kernel_spmd

    in_maps, n_pad, passes, use_b1 = make_in_maps(x, batch, W1, b1, W2, b2)
    key = (n_pad, passes, use_b1)
    nc = _PROGRAM_CACHE.get(key)
    if nc is None:
        nc = build_program(n_pad, passes, use_b1)
        _PROGRAM_CACHE[key] = nc
    res = run_bass_kernel_spmd(nc, in_maps, list(range(N_CORES)))
    outs = []
    for c in range(N_CORES):
        a = res.results[c]["out"]  # [G_LOC, NLANE, HIDDEN+1], lane-rotated
        total = np.zeros((G_LOC, HIDDEN + 1), np.float64)
        for j in range(NLANE):
            total += np.roll(a[:, j, :], -WIN * j, axis=0)
        outs.append(
            (total[:, :HIDDEN] / np.maximum(total[:, HIDDEN:], 1e-30)).astype(
                np.float32
            )
        )
    return np.concatenate(outs, axis=0)


# revision 35
# speedup vs baseline: 1.1919x; 1.1919x over previous
"""AttentionPooling (segment softmax + weighted segment sum) on 8 trn2 cores.

Math (per graph g): out[g] = sum_n softmax_g(s)_n * x[n] over nodes n with
batch[n] == g, where s = tanh(x @ W1 + b1) @ W2 + b2.

Key design points:
  * exp(s) cannot overflow fp32 -> accumulate unnormalized exp(s)*x and
    exp(s), divide once at the end.  b2 shifts every score equally and
    cancels in the softmax -> dropped entirely.
  * batch is sorted, so sharding by graph (128 graphs per core) gives each
    core one contiguous node range: pure data parallel, no collectives.
  * Pool = matmul with weighted one-hot lhsT st[n, g'] = e_n * (bl[n] == g')
    over a 32-graph window (M=32).  The 4 blocks of a chunk go to four
    DIFFERENT tile_position col groups (slot = (window + lane) % 4, one PSUM
    accumulator per lane) so they stream CONCURRENTLY on the PE's 32-col
    sub-arrays (~284ns for 4 blocks vs ~548ns serialized).  The final
    combine un-rotates with 7 partition-shifted DVE ops.
  * Scores are written at PASS-aligned PSUM columns (a block covering two
    windows emits its score twice - only ~9 extra N=1 matmuls total), so
    the whole one-hot build for a chunk is TWO DVE tensor_tensor ops with
    3D broadcast APs (is_equal vs bcols, multiply by ee) instead of ~9
    per-pass ops: DVE fixed overhead (~90-130ns/op) dominated the v1 build.
  * ONE Exp per super-chunk: ScalarE ACTIVATE costs (N+352)/1.2 ns, so
    batching 16+ scores per exp amortizes the 352-cycle fixed cost.
  * ~22 N=512 warmup matmuls on zeroed data keep the PE busy through the
    HAM activity window (~3.4us) during the initial DMA fill, so the clock
    gate is at 8/8 (2.4 GHz) when real work starts.
  * Both x streams are fp8 e3m4; W1 stays bf16 (rel err 1.46e-2 < 2e-2).
"""

import sys
from contextlib import ExitStack

import numpy as np

for _p in ("/opt/trn_rl_repo",):
    if _p not in sys.path:
        sys.path.insert(0, _p)

import ml_dtypes

import concourse.bass as bass
import concourse.bacc as bacc
import concourse.tile as tile
from concourse import mybir

N_NODES = 500_000
HIDDEN = 256
NUM_GRAPHS = 1024
N_CORES = 8
G_LOC = NUM_GRAPHS // N_CORES  # 128 graphs per core == PSUM partition dim
H = HIDDEN // 2  # 128 hidden units in the attention MLP
BLK = 128  # nodes per block (matmul contraction tile)
NBPC = 4  # blocks per chunk (also: pool rotation lanes)
CH = BLK * NBPC  # 512 nodes per compute chunk (one PSUM bank at fp32)
CPS = 4  # compute chunks per DMA super-chunk
SUP = CH * CPS  # 2048 nodes per DMA (~1 MB per stream -> efficient descriptors)
WIN = 32  # pool window: graphs per one-hot / PSUM col group
NLANE = 4  # pool rotation lanes == NBPC
BF16 = mybir.dt.bfloat16
E3M4 = mybir.dt.float8e3  # 4 mantissa bits: x streams (rel err ~3%, max ~15.5)
F32 = mybir.dt.float32

_PROGRAM_CACHE: dict = {}


def build_program(n_pad: int, passes: tuple, use_b1: bool) -> bass.Bass:
    """passes[blk] = tuple of 32-graph windows the block's pool matmul must
    cover (union across cores; usually 1, occasionally 2)."""
    assert n_pad % SUP == 0
    nblk = n_pad // BLK
    nsup = n_pad // SUP
    nchunks = n_pad // CH
    assert len(passes) == nblk

    # flat pass list [(blk, w, idx)] in emission order; per (lane, slot) the
    # first and last flat index (lane = blk % NLANE, slot = (w+lane) % NLANE)
    flat = []
    for blk in range(nblk):
        for w in passes[blk]:
            flat.append((blk, w, len(flat)))
    npass = len(flat)
    first = {}
    last = {}
    for blk, w, idx in flat:
        lane = blk % NLANE
        slot = (w + lane) % NLANE
        first.setdefault((lane, slot), idx)
        last[(lane, slot)] = idx
    pass_of_blk = {}
    for blk, w, idx in flat:
        pass_of_blk.setdefault(blk, []).append((w, idx))

    # per-chunk / per-super pass spans (flat indices are contiguous per chunk)
    def blk_range_passes(b0, b1):
        return [
            (blk, w, idx)
            for blk, w, idx in flat
            if b0 <= blk < b1
        ]

    chunk_passes = [blk_range_passes(t * NBPC, (t + 1) * NBPC) for t in range(nchunks)]
    sup_start = []
    for s in range(nsup):
        sp_list = blk_range_passes(s * NBPC * CPS, (s + 1) * NBPC * CPS)
        sup_start.append(sp_list[0][2] if sp_list else npass)
    sup_npass = [
        len(blk_range_passes(s * NBPC * CPS, (s + 1) * NBPC * CPS))
        for s in range(nsup)
    ]
    maxpc = max((len(cp) for cp in chunk_passes), default=1)
    supw = max(sup_npass) if sup_npass else 1
    assert supw <= WIN, f"super pass count {supw} exceeds sp tile width"

    nc = bacc.Bacc("TRN2")
    # host-swizzled so each super-chunk DMA reads one contiguous ~4KB run per
    # partition: xaug[s, p, b, f] = [x | 1.0][s*SUP + b*BLK + p, f]
    xaug = nc.dram_tensor(
        "xaug", [nsup, BLK, NBPC * CPS, HIDDEN + 1], E3M4, kind="ExternalInput"
    )
    # xT[s, p, j, n] = x[s*SUP + n, BLK*j + p], fp8: feeds only the score MLP
    xT = nc.dram_tensor("xT", [nsup, BLK, 2, SUP], E3M4, kind="ExternalInput")
    # bcols[p, pass] = batch_local[blk(pass)*BLK + p] - 32*w(pass)  (or pad)
    bcols = nc.dram_tensor("bcols", [BLK, max(npass, 1)], BF16, kind="ExternalInput")
    # w1[p, j, h] = W1[BLK*j + p, h]
    w1 = nc.dram_tensor("w1", [BLK, 2, H], BF16, kind="ExternalInput")
    w2 = nc.dram_tensor("w2", [H, 1], BF16, kind="ExternalInput")
    if use_b1:
        b1 = nc.dram_tensor("b1", [H, 1], F32, kind="ExternalInput")
    # raw rotated lane accumulators; the host un-rotates, sums lanes and
    # normalizes (cheap numpy) - saves ~5us of on-device tail work
    out = nc.dram_tensor(
        "out", [G_LOC, NLANE, HIDDEN + 1], F32, kind="ExternalOutput"
    )

    with tile.TileContext(nc) as tc, ExitStack() as ctx:
        singles = ctx.enter_context(tc.tile_pool(name="singles", bufs=1))
        xa_pool = ctx.enter_context(tc.tile_pool(name="xa", bufs=4))
        xt_pool = ctx.enter_context(tc.tile_pool(name="xt", bufs=4))
        tt_pool = ctx.enter_context(tc.tile_pool(name="tt", bufs=3))
        oh_pool = ctx.enter_context(tc.tile_pool(name="oh", bufs=4))
        st_pool = ctx.enter_context(tc.tile_pool(name="st", bufs=4))
        ee_pool = ctx.enter_context(tc.tile_pool(name="ee", bufs=2))
        hp_pool = ctx.enter_context(tc.tile_pool(name="hp", bufs=2, space="PSUM"))
        sp_pool = ctx.enter_context(tc.tile_pool(name="sp", bufs=2, space="PSUM"))
        acc_pool = ctx.enter_context(tc.tile_pool(name="acc", bufs=1, space="PSUM"))

        # singles go through ScalarE's HWDGE trigger queue so SyncE's first
        # (serial, ~600ns each) triggers are the xt[0] quarter fills
        w1_sb = singles.tile([BLK, 2, H], BF16)
        nc.scalar.dma_start(out=w1_sb, in_=w1[:, :, :])
        w2_sb = singles.tile([H, 1], BF16)
        nc.scalar.dma_start(out=w2_sb, in_=w2[:, :])
        bc_sb = singles.tile([BLK, max(npass, 1)], BF16)
        nc.scalar.dma_start(out=bc_sb, in_=bcols[:, :])
        if use_b1:
            b1_sb = singles.tile([H, 1], F32)
            nc.scalar.dma_start(out=b1_sb, in_=b1[:, :])
        # junk (warmup operand) is filled by a gpsimd iota, the earliest
        # available producer (~2.9us; the fp8 gpsimd memset used before took
        # until ~8us, and the DVE stream is blocked by its table-load DMA)
        junk = singles.tile([BLK, CH], BF16)
        nc.gpsimd.iota(
            out=junk,
            pattern=[[1, CH]],
            base=0,
            channel_multiplier=0,
            allow_small_or_imprecise_dtypes=True,
        )
        iota_sb = singles.tile([BLK, WIN], BF16)
        nc.gpsimd.iota(
            out=iota_sb,
            pattern=[[1, WIN]],
            base=0,
            channel_multiplier=0,
            allow_small_or_imprecise_dtypes=True,
        )

        # rotated pool accumulators: lane j accumulates window w at partition
        # slot 32*((w+j)%4) of accs[j]
        accs = [
            acc_pool.tile([G_LOC, HIDDEN + 1], F32, tag=f"acc{j}", name=f"acc{j}")
            for j in range(NLANE)
        ]
        # zero any (lane, slot) region no matmul will ever write (the combine
        # below reads whole accumulators)
        for j in range(NLANE):
            for s in range(NLANE):
                if (j, s) not in first:
                    nc.vector.memset(accs[j][WIN * s : WIN * (s + 1), :], 0.0)

        # ~36 N=512 warmup matmuls (~8us) keep the PE busy through the HAM
        # activity window while the first super-chunk DMAs land.  They only
        # depend on the DVE memset above, so they start at ~0.4us.
        warm = hp_pool.tile([H, CH], F32, tag="hp", name="hp_warm")
        for i in range(36):
            nc.tensor.matmul(
                warm[0:WIN, :],
                lhsT=junk[:, 0:WIN],
                rhs=junk,
                start=True,
                stop=True,
            )

        xa_tiles = {}
        xt_tiles = {}
        tt_tiles = {}
        sp_tiles = {}
        ee_tiles = {}
        st_tiles = {}

        def emit_mlp(t):
            if not chunk_passes[t]:
                return
            s, q = divmod(t, CPS)
            xt = xt_tiles[s]
            hp = hp_pool.tile([H, CH], F32, tag="hp", name="hp")
            nc.tensor.matmul(
                hp,
                lhsT=w1_sb[:, 0, :],
                rhs=xt[:, 0, q * CH : (q + 1) * CH],
                start=True,
                stop=False,
            )
            nc.tensor.matmul(
                hp,
                lhsT=w1_sb[:, 1, :],
                rhs=xt[:, 1, q * CH : (q + 1) * CH],
                start=False,
                stop=True,
            )
            tt = tt_pool.tile([H, CH], E3M4, name="tt")
            kw = {"bias": b1_sb} if use_b1 else {}
            nc.scalar.activation(
                out=tt, in_=hp, func=mybir.ActivationFunctionType.Tanh, **kw
            )
            tt_tiles[t] = tt

        def emit_scores(t):
            if t not in tt_tiles:
                return
            s, q = divmod(t, CPS)
            if s not in sp_tiles:
                sp_tiles[s] = sp_pool.tile([BLK, WIN], F32, tag="sp", name="sp")
            sp = sp_tiles[s]
            tt = tt_tiles.pop(t)
            for blk, w, idx in chunk_passes[t]:
                b = blk % NBPC
                c = idx - sup_start[s]
                nc.tensor.matmul(
                    sp[:, c : c + 1],
                    lhsT=tt[:, b * BLK : (b + 1) * BLK],
                    rhs=w2_sb,
                    start=True,
                    stop=True,
                )

        def emit_exp(s):
            if s not in sp_tiles:
                return
            sp = sp_tiles.pop(s)
            n = sup_npass[s]
            ee = ee_pool.tile([BLK, WIN], BF16, tag="ee", name="ee")
            nc.scalar.activation(
                out=ee[:, 0:n], in_=sp[:, 0:n], func=mybir.ActivationFunctionType.Exp
            )
            ee_tiles[s] = ee

        def emit_st(t):
            """Two batched DVE ops build all weighted one-hots of chunk t."""
            cp = chunk_passes[t]
            if not cp:
                return
            s = t // CPS
            ee = ee_tiles[s]
            npc = len(cp)
            i0 = cp[0][2]
            j0 = i0 - sup_start[s]
            oh = oh_pool.tile([BLK, maxpc, WIN], BF16, tag="oh", name="oh")
            nc.vector.tensor_tensor(
                out=oh[:, 0:npc, :],
                in0=iota_sb[:, :].unsqueeze(1).broadcast_to((BLK, npc, WIN)),
                in1=bc_sb[:, i0 : i0 + npc].unsqueeze(2).broadcast_to((BLK, npc, WIN)),
                op=mybir.AluOpType.is_equal,
            )
            st = st_pool.tile([BLK, maxpc, WIN], BF16, tag="st", name="st")
            nc.vector.tensor_tensor(
                out=st[:, 0:npc, :],
                in0=oh[:, 0:npc, :],
                in1=ee[:, j0 : j0 + npc].unsqueeze(2).broadcast_to((BLK, npc, WIN)),
                op=mybir.AluOpType.mult,
            )
            st_tiles[t] = st

        def emit_pool(t):
            cp = chunk_passes[t]
            if not cp:
                return
            s, q = divmod(t, CPS)
            xa = xa_tiles[s]
            if q == CPS - 1:
                xa_tiles.pop(s)
            st = st_tiles.pop(t)
            i0 = cp[0][2]
            for blk, w, idx in cp:
                lane = blk % NLANE
                slot = (w + lane) % NLANE
                nc.tensor.matmul(
                    accs[lane][WIN * slot : WIN * (slot + 1), :],
                    lhsT=st[:, idx - i0, :],
                    rhs=xa[:, blk % (NBPC * CPS), :],
                    start=(idx == first[(lane, slot)]),
                    stop=(idx == last[(lane, slot)]),
                    tile_position=(0, WIN * slot),
                )

        def dma_xt(s, split=False):
            xt = xt_pool.tile([BLK, 2, SUP], E3M4, name="xt")
            if split:
                # finer first fills so the MLP can start sooner
                for qq in range(CPS):
                    nc.sync.dma_start(
                        out=xt[:, :, qq * CH : (qq + 1) * CH],
                        in_=xT[s][:, :, qq * CH : (qq + 1) * CH],
                    )
            else:
                nc.sync.dma_start(out=xt, in_=xT[s])
            xt_tiles[s] = xt

        def dma_xa(s):
            xa = xa_pool.tile([BLK, NBPC * CPS, HIDDEN + 1], E3M4, name="xa")
            nc.sync.dma_start(out=xa, in_=xaug[s])
            xa_tiles[s] = xa

        for t in range(nchunks + 6):
            s, q = divmod(t, CPS)
            if t == 0:
                # prioritize the MLP's stream: xt[0] (split), xt[1], THEN
                # xa[0] (first needed 6 slots later) - the DMA queue is FIFO
                # and the ramp is bandwidth-bound.
                dma_xt(0, split=True)
                if nsup > 1:
                    dma_xt(1)
                dma_xa(0)
            elif q == 0 and 1 <= s < nsup:
                if s + 1 < nsup:
                    dma_xt(s + 1)
                dma_xa(s)
            if t < nchunks:
                emit_mlp(t)
            if 0 <= t - 6 < nchunks:
                emit_pool(t - 6)
            if 0 <= t - 1 < nchunks:
                emit_scores(t - 1)
                if (t - 1) % CPS == CPS - 1:
                    emit_exp((t - 1) // CPS)
                    xt_tiles.pop((t - 1) // CPS, None)
            if 0 <= t - 4 < nchunks:
                emit_st(t - 4)

        # copy each (rotated) lane accumulator to SBUF (DVE/ScalarE split for
        # parallelism) and DMA all four out in ONE transfer (a dma_start
        # trigger costs ~620ns on SyncE); the host un-rotates + normalizes.
        accsb = singles.tile([G_LOC, NLANE, HIDDEN + 1], F32)
        for j in range(NLANE):
            if j % 2 == 0:
                nc.vector.tensor_copy(out=accsb[:, j, :], in_=accs[j])
            else:
                nc.scalar.copy(out=accsb[:, j, :], in_=accs[j])
        nc.sync.dma_start(out=out[:, :, :], in_=accsb)

    nc.finalize()
    return nc


def make_in_maps(x, batch, W1, b1, W2, b2):
    """Shard by graph (128 contiguous graphs per core), pad node counts to a
    common multiple of SUP, and lay out the per-core device arrays.  Also
    derives the uniform (across cores) pool pass structure."""
    x = np.asarray(x, dtype=np.float32)
    batch = np.asarray(batch)
    bounds = np.searchsorted(batch, np.arange(0, NUM_GRAPHS + 1, G_LOC))
    n_loc_max = int(np.diff(bounds).max())
    n_pad = max(SUP, ((n_loc_max + SUP - 1) // SUP) * SUP)
    nblk = n_pad // BLK

    # local (per-core) batch ids, -1 padding
    bl_all = np.full((N_CORES, n_pad), -1.0, np.float32)
    for c in range(N_CORES):
        s, e = int(bounds[c]), int(bounds[c + 1])
        bl_all[c, : e - s] = batch[s:e].astype(np.float32) - np.float32(c * G_LOC)

    # uniform pass structure: per block, union of windows over cores
    passes = []
    for blk in range(nblk):
        seg = bl_all[:, blk * BLK : (blk + 1) * BLK]
        ws = sorted({int(g) // WIN for g in np.unique(seg) if g >= 0})
        passes.append(tuple(ws))
    passes = tuple(passes)

    flat = [(blk, w) for blk in range(nblk) for w in passes[blk]]
    npass = len(flat)

    # w1[p, j, h] = W1[BLK*j + p, h], bf16 (scores must stay clean: the e3m4
    # pool stream eats most of the error budget)
    w1_8 = np.ascontiguousarray(
        np.asarray(W1, np.float32)
        .astype(ml_dtypes.bfloat16)
        .reshape(2, BLK, H)
        .transpose(1, 0, 2)
    )
    w2_bf = np.asarray(W2, np.float32).reshape(H, 1).astype(ml_dtypes.bfloat16)
    b1_f = np.asarray(b1, np.float32).reshape(H, 1)
    use_b1 = bool(np.any(b1_f != 0.0))

    in_maps = []
    for c in range(N_CORES):
        s, e = int(bounds[c]), int(bounds[c + 1])
        nloc = e - s
        xs = x[s:e]
        nsup = n_pad // SUP
        nb = NBPC * CPS
        xa = np.zeros((n_pad, HIDDEN + 1), ml_dtypes.float8_e3m4)
        xa[:nloc, :HIDDEN] = xs.astype(ml_dtypes.float8_e3m4)
        xa[:nloc, HIDDEN] = 1.0
        # [s*SUP + b*BLK + p, f] -> [s, p, b, f]
        xa = np.ascontiguousarray(
            xa.reshape(nsup, nb, BLK, HIDDEN + 1).transpose(0, 2, 1, 3)
        )
        # [s, p, j, n] = x[s*SUP + n, BLK*j + p]
        xT = np.zeros((HIDDEN, n_pad), ml_dtypes.float8_e3m4)
        xT[:, :nloc] = xs.T.astype(ml_dtypes.float8_e3m4)
        xT = np.ascontiguousarray(xT.reshape(2, BLK, nsup, SUP).transpose(2, 1, 0, 3))
        bl = bl_all[c]
        bcols = np.full((BLK, max(npass, 1)), -1e9, np.float32)
        for i, (blk, w) in enumerate(flat):
            bcols[:, i] = bl[blk * BLK : (blk + 1) * BLK] - np.float32(WIN * w)
        im = {
            "xaug": xa,
            "xT": xT,
            "bcols": np.ascontiguousarray(bcols.astype(ml_dtypes.bfloat16)),
            "w1": w1_8,
            "w2": w2_bf,
        }
        if use_b1:
            im["b1"] = b1_f
        in_maps.append(im)
    return in_maps, n_pad, passes, use_b1


def kernel(x, batch, W1, b1, W2, b2):
    from concourse.bass_utils import run_bass_kernel_spmd

    in_maps, n_pad, passes, use_b1 = make_in_maps(x, batch, W1, b1, W2, b2)
    key = (n_pad, passes, use_b1)
    nc = _PROGRAM_CACHE.get(key)
    if nc is None:
        nc = build_program(n_pad, passes, use_b1)
        _PROGRAM_CACHE[key] = nc
    res = run_bass_kernel_spmd(nc, in_maps, list(range(N_CORES)))
    outs = []
    for c in range(N_CORES):
        a = res.results[c]["out"]  # [G_LOC, NLANE, HIDDEN+1], lane-rotated
        total = np.zeros((G_LOC, HIDDEN + 1), np.float64)
        for j in range(NLANE):
            total += np.roll(a[:, j, :], -WIN * j, axis=0)
        outs.append(
            (total[:, :HIDDEN] / np.maximum(total[:, HIDDEN:], 1e-30)).astype(
                np.float32
            )
        )
    return np.concatenate(outs, axis=0)


# revision 36
# speedup vs baseline: 1.2459x; 1.0454x over previous
"""AttentionPooling (segment softmax + weighted segment sum) on 8 trn2 cores.

Math (per graph g): out[g] = sum_n softmax_g(s)_n * x[n] over nodes n with
batch[n] == g, where s = tanh(x @ W1 + b1) @ W2 + b2.

Key design points:
  * exp(s) cannot overflow fp32 -> accumulate unnormalized exp(s)*x and
    exp(s), divide once at the end.  b2 shifts every score equally and
    cancels in the softmax -> dropped entirely.
  * batch is sorted, so sharding by graph (128 graphs per core) gives each
    core one contiguous node range: pure data parallel, no collectives.
  * Pool = matmul with weighted one-hot lhsT st[n, g'] = e_n * (bl[n] == g')
    over a 32-graph window (M=32).  The 4 blocks of a chunk go to four
    DIFFERENT tile_position col groups (slot = (window + lane) % 4, one PSUM
    accumulator per lane) so they stream CONCURRENTLY on the PE's 32-col
    sub-arrays (~284ns for 4 blocks vs ~548ns serialized).  The final
    combine un-rotates with 7 partition-shifted DVE ops.
  * Scores are written at PASS-aligned PSUM columns (a block covering two
    windows emits its score twice - only ~9 extra N=1 matmuls total), so
    the whole one-hot build for a chunk is TWO DVE tensor_tensor ops with
    3D broadcast APs (is_equal vs bcols, multiply by ee) instead of ~9
    per-pass ops: DVE fixed overhead (~90-130ns/op) dominated the v1 build.
  * ONE Exp per super-chunk: ScalarE ACTIVATE costs (N+352)/1.2 ns, so
    batching 16+ scores per exp amortizes the 352-cycle fixed cost.
  * ~22 N=512 warmup matmuls on zeroed data keep the PE busy through the
    HAM activity window (~3.4us) during the initial DMA fill, so the clock
    gate is at 8/8 (2.4 GHz) when real work starts.
  * Both x streams are fp8 e3m4; W1 stays bf16 (rel err 1.46e-2 < 2e-2).
"""

import sys
from contextlib import ExitStack

import numpy as np

for _p in ("/opt/trn_rl_repo",):
    if _p not in sys.path:
        sys.path.insert(0, _p)

import ml_dtypes

import concourse.bass as bass
import concourse.bacc as bacc
import concourse.tile as tile
from concourse import mybir

N_NODES = 500_000
HIDDEN = 256
NUM_GRAPHS = 1024
N_CORES = 8
G_LOC = NUM_GRAPHS // N_CORES  # 128 graphs per core == PSUM partition dim
H = HIDDEN // 2  # 128 hidden units in the attention MLP
BLK = 128  # nodes per block (matmul contraction tile)
NBPC = 4  # blocks per chunk (also: pool rotation lanes)
CH = BLK * NBPC  # 512 nodes per compute chunk (one PSUM bank at fp32)
CPS = 4  # compute chunks per DMA super-chunk
SUP = CH * CPS  # 2048 nodes per DMA (~1 MB per stream -> efficient descriptors)
WIN = 32  # pool window: graphs per one-hot / PSUM col group
NLANE = 4  # pool rotation lanes == NBPC
BF16 = mybir.dt.bfloat16
E3M4 = mybir.dt.float8e3  # 4 mantissa bits: x streams (rel err ~3%, max ~15.5)
F32 = mybir.dt.float32

_PROGRAM_CACHE: dict = {}


def build_program(n_pad: int, passes: tuple, use_b1: bool) -> bass.Bass:
    """passes[blk] = tuple of 32-graph windows the block's pool matmul must
    cover (union across cores; usually 1, occasionally 2)."""
    assert n_pad % SUP == 0
    nblk = n_pad // BLK
    nsup = n_pad // SUP
    nchunks = n_pad // CH
    assert len(passes) == nblk

    # flat pass list [(blk, w, idx)] in emission order; per (lane, slot) the
    # first and last flat index (lane = blk % NLANE, slot = (w+lane) % NLANE)
    flat = []
    for blk in range(nblk):
        for w in passes[blk]:
            flat.append((blk, w, len(flat)))
    npass = len(flat)
    first = {}
    last = {}
    for blk, w, idx in flat:
        lane = blk % NLANE
        slot = (w + lane) % NLANE
        first.setdefault((lane, slot), idx)
        last[(lane, slot)] = idx
    pass_of_blk = {}
    for blk, w, idx in flat:
        pass_of_blk.setdefault(blk, []).append((w, idx))

    # per-chunk / per-super pass spans (flat indices are contiguous per chunk)
    def blk_range_passes(b0, b1):
        return [
            (blk, w, idx)
            for blk, w, idx in flat
            if b0 <= blk < b1
        ]

    chunk_passes = [blk_range_passes(t * NBPC, (t + 1) * NBPC) for t in range(nchunks)]
    sup_start = []
    for s in range(nsup):
        sp_list = blk_range_passes(s * NBPC * CPS, (s + 1) * NBPC * CPS)
        sup_start.append(sp_list[0][2] if sp_list else npass)
    sup_npass = [
        len(blk_range_passes(s * NBPC * CPS, (s + 1) * NBPC * CPS))
        for s in range(nsup)
    ]
    maxpc = max((len(cp) for cp in chunk_passes), default=1)
    supw = max(sup_npass) if sup_npass else 1
    assert supw <= WIN, f"super pass count {supw} exceeds sp tile width"

    nc = bacc.Bacc("TRN2")
    # host-swizzled so each super-chunk DMA reads one contiguous ~4KB run per
    # partition: xaug[s, p, b, f] = [x | 1.0][s*SUP + b*BLK + p, f]
    xaug = nc.dram_tensor(
        "xaug", [nsup, BLK, NBPC * CPS, HIDDEN + 1], E3M4, kind="ExternalInput"
    )
    # xT[s, p, j, n] = x[s*SUP + n, BLK*j + p], fp8: feeds only the score MLP
    xT = nc.dram_tensor("xT", [nsup, BLK, 2, SUP], E3M4, kind="ExternalInput")
    # bcols[p, pass] = batch_local[blk(pass)*BLK + p] - 32*w(pass)  (or pad)
    bcols = nc.dram_tensor("bcols", [BLK, max(npass, 1)], BF16, kind="ExternalInput")
    # w1[p, j, h] = W1[BLK*j + p, h]
    w1 = nc.dram_tensor("w1", [BLK, 2, H], BF16, kind="ExternalInput")
    w2 = nc.dram_tensor("w2", [H, 1], BF16, kind="ExternalInput")
    if use_b1:
        b1 = nc.dram_tensor("b1", [H, 1], F32, kind="ExternalInput")
    # raw rotated lane accumulators; the host un-rotates, sums lanes and
    # normalizes (cheap numpy) - saves ~5us of on-device tail work
    out = nc.dram_tensor(
        "out", [G_LOC, NLANE, HIDDEN + 1], F32, kind="ExternalOutput"
    )

    with tile.TileContext(nc) as tc, ExitStack() as ctx:
        singles = ctx.enter_context(tc.tile_pool(name="singles", bufs=1))
        xa_pool = ctx.enter_context(tc.tile_pool(name="xa", bufs=4))
        xt_pool = ctx.enter_context(tc.tile_pool(name="xt", bufs=4))
        tt_pool = ctx.enter_context(tc.tile_pool(name="tt", bufs=3))
        oh_pool = ctx.enter_context(tc.tile_pool(name="oh", bufs=4))
        st_pool = ctx.enter_context(tc.tile_pool(name="st", bufs=4))
        ee_pool = ctx.enter_context(tc.tile_pool(name="ee", bufs=2))
        hp_pool = ctx.enter_context(tc.tile_pool(name="hp", bufs=2, space="PSUM"))
        sp_pool = ctx.enter_context(tc.tile_pool(name="sp", bufs=2, space="PSUM"))
        acc_pool = ctx.enter_context(tc.tile_pool(name="acc", bufs=1, space="PSUM"))

        # singles go through ScalarE's HWDGE trigger queue so SyncE's first
        # (serial, ~600ns each) triggers are the xt[0] quarter fills
        w1_sb = singles.tile([BLK, 2, H], BF16)
        nc.scalar.dma_start(out=w1_sb, in_=w1[:, :, :])
        w2_sb = singles.tile([H, 1], BF16)
        nc.scalar.dma_start(out=w2_sb, in_=w2[:, :])
        bc_sb = singles.tile([BLK, max(npass, 1)], BF16)
        nc.scalar.dma_start(out=bc_sb, in_=bcols[:, :])
        if use_b1:
            b1_sb = singles.tile([H, 1], F32)
            nc.scalar.dma_start(out=b1_sb, in_=b1[:, :])
        # memset on GpSimd: ready ~3us after the init barrier, while the DVE
        # stream is blocked until ~8us by its perf-mode table-load DMA
        junk = singles.tile([BLK, CH], E3M4)
        nc.gpsimd.memset(junk, 0.0)
        iota_sb = singles.tile([BLK, WIN], BF16)
        nc.gpsimd.iota(
            out=iota_sb,
            pattern=[[1, WIN]],
            base=0,
            channel_multiplier=0,
            allow_small_or_imprecise_dtypes=True,
        )

        # rotated pool accumulators: lane j accumulates window w at partition
        # slot 32*((w+j)%4) of accs[j]
        accs = [
            acc_pool.tile([G_LOC, HIDDEN + 1], F32, tag=f"acc{j}", name=f"acc{j}")
            for j in range(NLANE)
        ]
        # zero any (lane, slot) region no matmul will ever write (the combine
        # below reads whole accumulators)
        for j in range(NLANE):
            for s in range(NLANE):
                if (j, s) not in first:
                    nc.vector.memset(accs[j][WIN * s : WIN * (s + 1), :], 0.0)

        # ~14 N=512 warmup matmuls (~3us) keep the PE busy through the HAM
        # activity window while the first super-chunk DMAs land.  They only
        # depend on the DVE memset above, so they start at ~0.4us.
        warm = hp_pool.tile([H, CH], F32, tag="hp", name="hp_warm")
        for i in range(14):
            nc.tensor.matmul(
                warm[0:WIN, :],
                lhsT=junk[:, 0:WIN],
                rhs=junk,
                start=True,
                stop=True,
            )

        xa_tiles = {}
        xt_tiles = {}
        tt_tiles = {}
        sp_tiles = {}
        ee_tiles = {}
        st_tiles = {}

        def emit_mlp(t):
            if not chunk_passes[t]:
                return
            s, q = divmod(t, CPS)
            xt = xt_tiles[s]
            hp = hp_pool.tile([H, CH], F32, tag="hp", name="hp")
            nc.tensor.matmul(
                hp,
                lhsT=w1_sb[:, 0, :],
                rhs=xt[:, 0, q * CH : (q + 1) * CH],
                start=True,
                stop=False,
            )
            nc.tensor.matmul(
                hp,
                lhsT=w1_sb[:, 1, :],
                rhs=xt[:, 1, q * CH : (q + 1) * CH],
                start=False,
                stop=True,
            )
            tt = tt_pool.tile([H, CH], E3M4, name="tt")
            kw = {"bias": b1_sb} if use_b1 else {}
            nc.scalar.activation(
                out=tt, in_=hp, func=mybir.ActivationFunctionType.Tanh, **kw
            )
            tt_tiles[t] = tt

        def emit_scores(t):
            if t not in tt_tiles:
                return
            s, q = divmod(t, CPS)
            if s not in sp_tiles:
                sp_tiles[s] = sp_pool.tile([BLK, WIN], F32, tag="sp", name="sp")
            sp = sp_tiles[s]
            tt = tt_tiles.pop(t)
            for blk, w, idx in chunk_passes[t]:
                b = blk % NBPC
                c = idx - sup_start[s]
                nc.tensor.matmul(
                    sp[:, c : c + 1],
                    lhsT=tt[:, b * BLK : (b + 1) * BLK],
                    rhs=w2_sb,
                    start=True,
                    stop=True,
                )

        def emit_exp(s):
            if s not in sp_tiles:
                return
            sp = sp_tiles.pop(s)
            n = sup_npass[s]
            ee = ee_pool.tile([BLK, WIN], BF16, tag="ee", name="ee")
            nc.scalar.activation(
                out=ee[:, 0:n], in_=sp[:, 0:n], func=mybir.ActivationFunctionType.Exp
            )
            ee_tiles[s] = ee

        def emit_st(t):
            """Two batched DVE ops build all weighted one-hots of chunk t."""
            cp = chunk_passes[t]
            if not cp:
                return
            s = t // CPS
            ee = ee_tiles[s]
            npc = len(cp)
            i0 = cp[0][2]
            j0 = i0 - sup_start[s]
            oh = oh_pool.tile([BLK, maxpc, WIN], BF16, tag="oh", name="oh")
            nc.vector.tensor_tensor(
                out=oh[:, 0:npc, :],
                in0=iota_sb[:, :].unsqueeze(1).broadcast_to((BLK, npc, WIN)),
                in1=bc_sb[:, i0 : i0 + npc].unsqueeze(2).broadcast_to((BLK, npc, WIN)),
                op=mybir.AluOpType.is_equal,
            )
            st = st_pool.tile([BLK, maxpc, WIN], BF16, tag="st", name="st")
            nc.vector.tensor_tensor(
                out=st[:, 0:npc, :],
                in0=oh[:, 0:npc, :],
                in1=ee[:, j0 : j0 + npc].unsqueeze(2).broadcast_to((BLK, npc, WIN)),
                op=mybir.AluOpType.mult,
            )
            st_tiles[t] = st

        def emit_pool(t):
            cp = chunk_passes[t]
            if not cp:
                return
            s, q = divmod(t, CPS)
            xa = xa_tiles[s]
            if q == CPS - 1:
                xa_tiles.pop(s)
            st = st_tiles.pop(t)
            i0 = cp[0][2]
            for blk, w, idx in cp:
                lane = blk % NLANE
                slot = (w + lane) % NLANE
                nc.tensor.matmul(
                    accs[lane][WIN * slot : WIN * (slot + 1), :],
                    lhsT=st[:, idx - i0, :],
                    rhs=xa[:, blk % (NBPC * CPS), :],
                    start=(idx == first[(lane, slot)]),
                    stop=(idx == last[(lane, slot)]),
                    tile_position=(0, WIN * slot),
                )

        def dma_xt(s, split=False):
            xt = xt_pool.tile([BLK, 2, SUP], E3M4, name="xt")
            if split:
                # finer first fills so the MLP can start sooner
                for qq in range(CPS):
                    nc.sync.dma_start(
                        out=xt[:, :, qq * CH : (qq + 1) * CH],
                        in_=xT[s][:, :, qq * CH : (qq + 1) * CH],
                    )
            else:
                nc.sync.dma_start(out=xt, in_=xT[s])
            xt_tiles[s] = xt

        def dma_xa(s):
            xa = xa_pool.tile([BLK, NBPC * CPS, HIDDEN + 1], E3M4, name="xa")
            nc.sync.dma_start(out=xa, in_=xaug[s])
            xa_tiles[s] = xa

        for t in range(nchunks + 6):
            s, q = divmod(t, CPS)
            if t == 0:
                # prioritize the MLP's stream: xt[0] (split), xt[1], THEN
                # xa[0] (first needed 6 slots later) - the DMA queue is FIFO
                # and the ramp is bandwidth-bound.
                dma_xt(0, split=True)
                if nsup > 1:
                    dma_xt(1)
                dma_xa(0)
            elif q == 0 and 1 <= s < nsup:
                if s + 1 < nsup:
                    dma_xt(s + 1)
                dma_xa(s)
            if t < nchunks:
                emit_mlp(t)
            if 0 <= t - 6 < nchunks:
                emit_pool(t - 6)
            if 0 <= t - 1 < nchunks:
                emit_scores(t - 1)
                if (t - 1) % CPS == CPS - 1:
                    emit_exp((t - 1) // CPS)
                    xt_tiles.pop((t - 1) // CPS, None)
            if 0 <= t - 4 < nchunks:
                emit_st(t - 4)

        # copy each (rotated) lane accumulator to SBUF (DVE/ScalarE split for
        # parallelism) and DMA all four out in ONE transfer (a dma_start
        # trigger costs ~620ns on SyncE); the host un-rotates + normalizes.
        accsb = singles.tile([G_LOC, NLANE, HIDDEN + 1], F32)
        for j in range(NLANE):
            if j % 2 == 0:
                nc.vector.tensor_copy(out=accsb[:, j, :], in_=accs[j])
            else:
                nc.scalar.copy(out=accsb[:, j, :], in_=accs[j])
        nc.sync.dma_start(out=out[:, :, :], in_=accsb)

    nc.finalize()
    return nc


def make_in_maps(x, batch, W1, b1, W2, b2):
    """Shard by graph (128 contiguous graphs per core), pad node counts to a
    common multiple of SUP, and lay out the per-core device arrays.  Also
    derives the uniform (across cores) pool pass structure."""
    x = np.asarray(x, dtype=np.float32)
    batch = np.asarray(batch)
    bounds = np.searchsorted(batch, np.arange(0, NUM_GRAPHS + 1, G_LOC))
    n_loc_max = int(np.diff(bounds).max())
    n_pad = max(SUP, ((n_loc_max + SUP - 1) // SUP) * SUP)
    nblk = n_pad // BLK

    # local (per-core) batch ids, -1 padding
    bl_all = np.full((N_CORES, n_pad), -1.0, np.float32)
    for c in range(N_CORES):
        s, e = int(bounds[c]), int(bounds[c + 1])
        bl_all[c, : e - s] = batch[s:e].astype(np.float32) - np.float32(c * G_LOC)

    # uniform pass structure: per block, union of windows over cores
    passes = []
    for blk in range(nblk):
        seg = bl_all[:, blk * BLK : (blk + 1) * BLK]
        ws = sorted({int(g) // WIN for g in np.unique(seg) if g >= 0})
        passes.append(tuple(ws))
    passes = tuple(passes)

    flat = [(blk, w) for blk in range(nblk) for w in passes[blk]]
    npass = len(flat)

    # w1[p, j, h] = W1[BLK*j + p, h], bf16 (scores must stay clean: the e3m4
    # pool stream eats most of the error budget)
    w1_8 = np.ascontiguousarray(
        np.asarray(W1, np.float32)
        .astype(ml_dtypes.bfloat16)
        .reshape(2, BLK, H)
        .transpose(1, 0, 2)
    )
    w2_bf = np.asarray(W2, np.float32).reshape(H, 1).astype(ml_dtypes.bfloat16)
    b1_f = np.asarray(b1, np.float32).reshape(H, 1)
    use_b1 = bool(np.any(b1_f != 0.0))

    in_maps = []
    for c in range(N_CORES):
        s, e = int(bounds[c]), int(bounds[c + 1])
        nloc = e - s
        xs = x[s:e]
        nsup = n_pad // SUP
        nb = NBPC * CPS
        xa = np.zeros((n_pad, HIDDEN + 1), ml_dtypes.float8_e3m4)
        xa[:nloc, :HIDDEN] = xs.astype(ml_dtypes.float8_e3m4)
        xa[:nloc, HIDDEN] = 1.0
        # [s*SUP + b*BLK + p, f] -> [s, p, b, f]
        xa = np.ascontiguousarray(
            xa.reshape(nsup, nb, BLK, HIDDEN + 1).transpose(0, 2, 1, 3)
        )
        # [s, p, j, n] = x[s*SUP + n, BLK*j + p]
        xT = np.zeros((HIDDEN, n_pad), ml_dtypes.float8_e3m4)
        xT[:, :nloc] = xs.T.astype(ml_dtypes.float8_e3m4)
        xT = np.ascontiguousarray(xT.reshape(2, BLK, nsup, SUP).transpose(2, 1, 0, 3))
        bl = bl_all[c]
        bcols = np.full((BLK, max(npass, 1)), -1e9, np.float32)
        for i, (blk, w) in enumerate(flat):
            bcols[:, i] = bl[blk * BLK : (blk + 1) * BLK] - np.float32(WIN * w)
        im = {
            "xaug": xa,
            "xT": xT,
            "bcols": np.ascontiguousarray(bcols.astype(ml_dtypes.bfloat16)),
            "w1": w1_8,
            "w2": w2_bf,
        }
        if use_b1:
            im["b1"] = b1_f
        in_maps.append(im)
    return in_maps, n_pad, passes, use_b1


def kernel(x, batch, W1, b1, W2, b2):
    from concourse.bass_utils import run_bass_kernel_spmd

    in_maps, n_pad, passes, use_b1 = make_in_maps(x, batch, W1, b1, W2, b2)
    key = (n_pad, passes, use_b1)
    nc = _PROGRAM_CACHE.get(key)
    if nc is None:
        nc = build_program(n_pad, passes, use_b1)
        _PROGRAM_CACHE[key] = nc
    res = run_bass_kernel_spmd(nc, in_maps, list(range(N_CORES)))
    outs = []
    for c in range(N_CORES):
        a = res.results[c]["out"]  # [G_LOC, NLANE, HIDDEN+1], lane-rotated
        total = np.zeros((G_LOC, HIDDEN + 1), np.float64)
        for j in range(NLANE):
            total += np.roll(a[:, j, :], -WIN * j, axis=0)
        outs.append(
            (total[:, :HIDDEN] / np.maximum(total[:, HIDDEN:], 1e-30)).astype(
                np.float32
            )
        )
    return np.concatenate(outs, axis=0)
